# revision 34
# baseline (speedup 1.0000x reference)
"""GAT message-passing kernel for 8 Trainium2 NeuronCores (Bass/Tile).

Strategy (edge-parallel with dst-routing, per the sharding hint):
  * Sort edges by destination node on the host; partition destination
    nodes into 8 contiguous, 128-aligned ranges (50 blocks of 128 nodes
    per core).  Each core owns the full segment-softmax + scatter-add
    for its dst range, so no cross-core collectives are needed.
  * Phase A (on device, replicated): project x -> [k||v] for all nodes
    into an HBM table (k stored fp16, v stored as bf16 bit-pattern in
    the same fp16-typed rows); project x -> q (fp16) for the core's
    local nodes.
  * Phase B (per 2-block superblock): dma_gather 512-byte k||v rows by
    edge src (lo/hi tables, int16 gather indices) and 256-byte q rows
    by edge dst.  All per-edge math is fused into a handful of wide
    per-superblock DVE/ACT instructions:
      P    = is_equal(iota_bcast, dstloc_bcast)          (one-hot, bf16)
      qk   = qg * kg                                     (fp16)
      s4   = reduce_d(qk)                                (f32 scores)
      e    = exp(s4)                                     (ACT, f32)
      msge = [v * e | e]                                 (bf16)
    then one PE matmul per 128-edge chunk accumulates agg/denom per
    block in PSUM.  Epilogue: agg/denom, transpose, @Wout + bias, relu,
    +x (f32), store.

The single Bass program is shared by all 8 cores (SPMD); everything
data-dependent (gather indices, dst-local ids) arrives as per-core
input tensors with uniform shapes.
"""

import math
import numpy as np
import ml_dtypes

# ----- problem constants (hardcoded per contest rules) -----
N = 50000
E = 800000
D = 128          # IN_DIM == OUT_DIM == HEADS*HEAD_DIM
H = 4
HD = 32
BLK = 128
LO_LIMIT = 32768  # int16 gather-index limit

BF16 = ml_dtypes.bfloat16
FP16 = np.float16


def _ceil_div(a, b):
    return (a + b - 1) // b


def _wrap16(stream_i16):
    """Pack a flat descriptor-index stream into the [128, n/16] SBUF layout
    dma_gather expects (idx i at [i%16, i//16], replicated to all 8
    16-partition groups)."""
    n = stream_i16.shape[0]
    assert n % 16 == 0
    a = stream_i16.reshape(n // 16, 16).T  # [16, n/16]
    return np.tile(a, (8, 1)).astype(np.int16)  # [128, n/16]


def _prep(x, edge_index, Wt, Ws, Wc, Wout, bout, ncores, nbc, sbb, lo_limit):
    """Host-side preprocessing: sort/route edges, build all per-core arrays.

    Returns (meta dict, list of per-core in_maps)."""
    npb = ncores * nbc              # total padded blocks
    npad = npb * BLK                # padded node count
    nnc = nbc * BLK                 # nodes per core
    nsb = nbc // sbb                # superblocks per core
    assert nbc % sbb == 0

    x = np.asarray(x, np.float32)
    n = x.shape[0]
    src = np.asarray(edge_index[0]).astype(np.int64)
    dst = np.asarray(edge_index[1]).astype(np.int64)

    # --- degree-balanced node->block assignment (cuts worst-case chunk
    # counts, i.e. gather-descriptor padding).  Output rows come back in
    # block order and are inverse-permuted on the host.  The lo stream is
    # q-PAIRED (2 edges of one dst share a q descriptor), so balance lo
    # CELLS = ceil(deg_lo/2); the hi stream stays per-edge. ---
    lo_cnt = np.bincount(dst[src < lo_limit], minlength=npad).astype(np.int64)
    hi_cnt = np.bincount(dst[src >= lo_limit], minlength=npad).astype(np.int64)
    lo_cells_n = (lo_cnt + 1) // 2
    tot = lo_cnt + hi_cnt
    node_order_desc = np.argsort(-tot, kind="stable")
    blk_lo = np.zeros(npb, np.int64)
    blk_hi = np.zeros(npb, np.int64)
    blk_n = np.zeros(npb, np.int64)
    blk_of = np.zeros(npad, np.int64)
    # weights normalise lo/hi fill fractions
    wl = 1.0 / max(lo_cells_n.sum() / npb, 1.0)
    wh = 1.0 / max(hi_cnt.sum() / npb, 1.0)
    FULL = 1 << 40
    score = np.zeros(npb, np.float64)
    for nd in node_order_desc:
        b = int(np.argmin(score))
        blk_of[nd] = b
        blk_lo[b] += lo_cells_n[nd]
        blk_hi[b] += hi_cnt[nd]
        blk_n[b] += 1
        score[b] = blk_lo[b] * wl + blk_hi[b] * wh + (FULL if blk_n[b] >= BLK else 0)
    # node list per block (block-major permutation)
    perm = np.argsort(blk_of * (npad + 1) + np.arange(npad), kind="stable")
    # local index of each node within its block
    loc_of = np.zeros(npad, np.int64)
    loc_of[perm] = np.arange(npad) % BLK

    eb = blk_of[dst]                 # block of each edge
    order = np.argsort(eb, kind="stable")
    src_s = src[order].astype(np.int32)
    dst_s = dst[order].astype(np.int32)
    eb_s = eb[order]
    bounds = np.searchsorted(eb_s, np.arange(npb + 1)).astype(np.int64)
    lo_list, hi_list = [], []   # per block: (src_idx, dst_local)
    for b in range(npb):
        s, e = bounds[b], bounds[b + 1]
        bs, bd = src_s[s:e], loc_of[dst_s[s:e]].astype(np.int32)
        m = bs < lo_limit
        lo_list.append((bs[m], bd[m]))
        hi_list.append((bs[~m] - lo_limit, bd[~m]))

    # lo stream layout: full pairs (2 edges of one dst share a q descriptor)
    # + a small flat section for odd leftovers; hi stream flat.  The q rows
    # are 512B [q | onehot(local id)] so the gather also delivers the
    # scatter one-hot column (no DVE P-build); pad slots point at a
    # dedicated zero-onehot row ZR.
    def _npairs(ld):
        if len(ld) == 0:
            return 0
        return int((np.bincount(ld) // 2).sum())

    def _nleft(ld):
        if len(ld) == 0:
            return 0
        return int((np.bincount(ld) % 2).sum())

    plo = _ceil_div(max(1, max(_npairs(a[1]) for a in lo_list)), BLK)
    flo = _ceil_div(max(_nleft(a[1]) for a in lo_list), BLK)
    cpb_lo = 2 * plo + flo          # lo chunks per block
    cpb_hi = max(_ceil_div(len(a[0]), BLK) for a in hi_list)
    ch = sbb * (cpb_lo + cpb_hi)    # chunks per superblock
    qcols = sbb * (plo + flo + cpb_hi)  # q-gather columns per superblock
    wlo = sbb * cpb_lo * BLK        # lo edges slots per superblock
    whi = sbb * cpb_hi * BLK
    fl0 = sbb * 2 * plo             # first flat-lo chunk
    h0 = sbb * cpb_lo               # first hi chunk

    # padded x / weights
    xpad = np.zeros((npad, D), np.float32)
    xpad[:n] = x
    xT16 = np.ascontiguousarray(xpad.T).astype(FP16)       # [D, npad] fp16
    Wskvc = np.ascontiguousarray(
        np.concatenate([np.asarray(Ws, np.float32), np.asarray(Wc, np.float32)],
                       axis=1)).astype(FP16)               # [D, 2D]
    iota = np.tile(np.arange(BLK, dtype=np.float32)[None, :], (BLK, 1)).astype(BF16)
    ident = np.eye(BLK, dtype=np.float32)
    bias_rep = np.tile(np.asarray(bout, np.float32)[None, :], (BLK, 1))

    in_maps = []
    for c in range(ncores):
        perm_core = perm[c * nnc:(c + 1) * nnc]
        kvlo = np.zeros((nsb, wlo), np.int16)
        kvhi = np.zeros((nsb, max(whi, 16)), np.int16)
        qidx = np.zeros((nsb, qcols * BLK), np.int16)
        dstl = np.full((nsb, ch * BLK), -1, np.int32)
        for s in range(nsb):
            for bb in range(sbb):
                gb = c * nbc + s * sbb + bb            # global block
                gloc = (gb - c * nbc) * BLK            # q-table row base
                (ls, ld), (hs, hd_) = lo_list[gb], hi_list[gb]
                # --- lo segment: full pairs + odd leftovers ---
                o = np.argsort(ld, kind="stable")
                ls_s, ld_s = ls[o], ld[o]
                if len(ld_s):
                    first = np.searchsorted(ld_s, ld_s)      # run starts
                    r = np.arange(len(ld_s)) - first         # pos in run
                    degs = np.bincount(ld_s)
                    pairs_pfx = np.concatenate([[0], np.cumsum(degs // 2)])
                    left_pfx = np.concatenate([[0], np.cumsum(degs % 2)])
                    is_pair = r < 2 * (degs[ld_s] // 2)
                    # paired edges
                    kp = pairs_pfx[ld_s[is_pair]] + r[is_pair] // 2
                    t = r[is_pair] % 2
                    u, p = kp // BLK, kp % BLK
                    slot = (bb * 2 * plo + 2 * u + t) * BLK + p
                    kvlo[s, slot] = ls_s[is_pair]
                    dstl[s, slot] = ld_s[is_pair]
                    qslot = (bb * plo + u) * BLK + p
                    qidx[s, qslot] = (gloc + ld_s[is_pair]).astype(np.int16)
                    # leftover edges (flat-lo section)
                    il = left_pfx[ld_s[~is_pair]]
                    fu, fp_ = il // BLK, il % BLK
                    slot = (fl0 + bb * flo + fu) * BLK + fp_
                    kvlo[s, slot] = ls_s[~is_pair]
                    dstl[s, slot] = ld_s[~is_pair]
                    qslot = (sbb * plo + bb * flo + fu) * BLK + fp_
                    qidx[s, qslot] = (gloc + ld_s[~is_pair]).astype(np.int16)
                # --- hi segment (flat, per-edge q) ---
                o = bb * cpb_hi * BLK
                kvhi[s, o:o + len(hs)] = hs
                do = (h0 + bb * cpb_hi) * BLK
                dstl[s, do:do + len(hd_)] = hd_
                qo = (sbb * (plo + flo) + bb * cpb_hi) * BLK
                qidx[s, qo:qo + len(hd_)] = (gloc + hd_).astype(np.int16)
        # wrap idx streams into dma_gather SBUF layout, concat over superblocks
        kvlo_w = np.concatenate([_wrap16(kvlo[s]) for s in range(nsb)], axis=1)
        kvhi_w = np.concatenate(
            [_wrap16(kvhi[s]) for s in range(nsb)], axis=1) if cpb_hi else \
            np.zeros((128, 16), np.int16)
        qidx_w = np.concatenate([_wrap16(qidx[s]) for s in range(nsb)], axis=1)
        dstl_w = np.ascontiguousarray(
            dstl.reshape(nsb * ch, BLK).T.astype(np.float32)).astype(BF16)

        in_maps.append({
            "dstloc": dstl_w,
            "iota": iota,
            "xT_full": xT16,
            "xT_local": np.ascontiguousarray(xT16[:, perm_core]),
            "x_local": np.ascontiguousarray(xpad[perm_core]),
            "kvlo_idx": np.ascontiguousarray(kvlo_w),
            "kvhi_idx": np.ascontiguousarray(kvhi_w),
            "q_idx": np.ascontiguousarray(qidx_w),
            "Wskvc": Wskvc,
            "Wt": np.ascontiguousarray(np.asarray(Wt, np.float32)).astype(FP16),
            "Wout": np.ascontiguousarray(np.asarray(Wout, np.float32)).astype(FP16),
            "identb": np.eye(BLK, dtype=np.float32).astype(BF16),
            "ident": ident,
            "bias_rep": bias_rep,
        })

    meta = dict(ncores=ncores, nbc=nbc, sbb=sbb, nsb=nsb, npb=npb, npad=npad,
                nnc=nnc, cpb_lo=cpb_lo, cpb_hi=cpb_hi, ch=ch, qcols=qcols,
                plo=plo, flo=flo, wlo=wlo, whi=whi, lo_limit=lo_limit, n=n,
                perm=perm)
    return meta, in_maps


def _build(meta):
    """Build the (single, SPMD-shared) Bass program."""
    from contextlib import ExitStack
    import concourse.bacc as bacc
    import concourse.mybir as mybir
    import concourse.tile as tile
    from concourse.tile import add_dep_helper

    f32 = mybir.dt.float32
    f16 = mybir.dt.float16
    bf16 = mybir.dt.bfloat16
    i16 = mybir.dt.int16
    Alu = mybir.AluOpType
    Act = mybir.ActivationFunctionType

    nbc, sbb, nsb = meta["nbc"], meta["sbb"], meta["nsb"]
    npb, npad, nnc = meta["npb"], meta["npad"], meta["nnc"]
    cpb_lo, cpb_hi, ch = meta["cpb_lo"], meta["cpb_hi"], meta["ch"]
    qcols, plo, flo = meta["qcols"], meta["plo"], meta["flo"]
    wlo, whi, lo_limit = meta["wlo"], meta["whi"], meta["lo_limit"]

    import os
    NQ = int(os.environ.get("K_NQUEUES", "1"))
    SP = bool(int(os.environ.get("K_SP", "0")))
    nc = bacc.Bacc("TRN2", target_bir_lowering=False, debug=False,
                   num_swdge_queues=NQ)

    t_xT = nc.dram_tensor("xT_full", [D, npad], f16, kind="ExternalInput")
    t_xTl = nc.dram_tensor("xT_local", [D, nnc], f16, kind="ExternalInput")
    t_xl = nc.dram_tensor("x_local", [nnc, D], f32, kind="ExternalInput")
    t_kvlo = nc.dram_tensor("kvlo_idx", [128, nsb * wlo // 16], i16,
                            kind="ExternalInput")
    t_kvhi = nc.dram_tensor("kvhi_idx", [128, max(nsb * whi // 16, 16)], i16,
                            kind="ExternalInput")
    t_qidx = nc.dram_tensor("q_idx", [128, nsb * qcols * BLK // 16], i16,
                            kind="ExternalInput")
    t_dstl = nc.dram_tensor("dstloc", [128, nsb * ch], bf16,
                            kind="ExternalInput")
    t_iota = nc.dram_tensor("iota", [BLK, BLK], bf16, kind="ExternalInput")
    t_wskvc = nc.dram_tensor("Wskvc", [D, 2 * D], f16, kind="ExternalInput")
    t_wt = nc.dram_tensor("Wt", [D, D], f16, kind="ExternalInput")
    t_wout = nc.dram_tensor("Wout", [D, D], f16, kind="ExternalInput")
    t_identb = nc.dram_tensor("identb", [BLK, BLK], bf16, kind="ExternalInput")
    t_ident = nc.dram_tensor("ident", [BLK, BLK], f32, kind="ExternalInput")
    t_bias = nc.dram_tensor("bias_rep", [BLK, BLK], f32, kind="ExternalInput")

    t_kv = nc.dram_tensor("kv_table", [npad, 2 * D], f16, kind="Internal")
    t_q = nc.dram_tensor("q_core", [nnc + 16, D], f16, kind="Internal")
    t_out = nc.dram_tensor("out", [nnc, D], f32, kind="ExternalOutput")

    store_insts = []

    with ExitStack() as ctx:
        tc = ctx.enter_context(tile.TileContext(nc))
        cpool = ctx.enter_context(tc.tile_pool(name="const", bufs=1))

        def load_const(t, shape, dtype):
            s = cpool.tile(shape, dtype, tag=t.name)
            nc.sync.dma_start(s[:], t[:])
            return s

        c_wskvc = load_const(t_wskvc, [D, 2 * D], f16)
        c_wt = load_const(t_wt, [D, D], f16)
        c_wout = load_const(t_wout, [D, D], f16)
        c_identb = load_const(t_identb, [BLK, BLK], bf16)
        c_ident = load_const(t_ident, [BLK, BLK], f32)
        c_bias = load_const(t_bias, [BLK, BLK], f32)
        c_kvlo = load_const(t_kvlo, list(t_kvlo.shape), i16)
        c_kvhi = load_const(t_kvhi, list(t_kvhi.shape), i16)
        c_qidx = load_const(t_qidx, list(t_qidx.shape), i16)
        c_dstl = load_const(t_dstl, list(t_dstl.shape), bf16)
        c_iota = load_const(t_iota, [BLK, BLK], bf16)

        # ---------------- Phase A: projections (fp16) ----------------
        # Big tiles / few DMAs: per iteration 2048 xT columns in one load,
        # 16 node-blocks of [k||v] out in ONE 1 MB store.
        with tc.tile_pool(name="pa", bufs=2) as pa, \
             tc.tile_pool(name="pa_ps", bufs=2, space="PSUM") as pa_ps:
            # q for local blocks first
            pos = 0
            sq = 0
            while pos < nnc:
                w = min(512, nnc - pos)
                nb = w // 128
                xq = pa.tile([128, 512], f16, tag="xq")
                nc.sync.dma_start(xq[:, 0:w], t_xTl[:, pos:pos + w])
                psq = pa_ps.tile([128, 512], f32, tag="qps")
                for b in range(nb):
                    nc.tensor.matmul(psq[:, b * 128:(b + 1) * 128],
                                     xq[:, b * 128:(b + 1) * 128],
                                     c_wt[:], start=True, stop=True)
                qsb = pa.tile([128, 4, D], f16, tag="qsb")
                if sq % 2 == 0:
                    nc.vector.tensor_copy(qsb[:, 0:nb, :], psq[:, 0:w])
                else:
                    nc.scalar.copy(qsb[:, 0:nb, :], psq[:, 0:w])
                st = nc.sync.dma_start(
                    t_q[pos:pos + w, :].rearrange("(b p) d -> p b d", b=nb),
                    qsb[:, 0:nb, :])
                store_insts.append(st.ins)
                pos += w
                sq += 1
            zt = pa.tile([16, D], f16, tag="zrow")
            nc.vector.memset(zt[:], 0.0)
            st = nc.sync.dma_start(t_q[nnc:nnc + 16, :], zt[:])
            store_insts.append(st.ins)

            # kv table for all nodes
            for g in range(npad // 2048):
                xa = pa.tile([128, 2048], f16, tag="xa")
                nc.sync.dma_start(xa[:], t_xT[:, g * 2048:(g + 1) * 2048])
                kvsb = pa.tile([128, 16, 256], f16, tag="kvsb")
                for k in range(4):   # quads of 128-node blocks
                    ps = pa_ps.tile([128, 4, 256], f32, tag="kvps")
                    for b in range(4):
                        nc.tensor.matmul(
                            ps[:, b, :],
                            xa[:, (4 * k + b) * BLK:(4 * k + b + 1) * BLK],
                            c_wskvc[:], start=True, stop=True)
                    kvv = kvsb[:, 4 * k:4 * k + 4, :].rearrange(
                        "p b (two d) -> p b two d", two=2)
                    psv = ps[:].rearrange("p b (two d) -> p b two d", two=2)
                    # k halves fp16 (DVE), v halves bf16 bit-pattern (ACT)
                    nc.vector.tensor_copy(kvv[:, :, 0, :], psv[:, :, 0, :])
                    nc.scalar.copy(kvv[:, :, 1, :].bitcast(bf16),
                                   psv[:, :, 1, :])
                st = nc.sync.dma_start(
                    t_kv[g * 2048:(g + 1) * 2048, :].rearrange(
                        "(b p) d -> p b d", b=16), kvsb[:])
                store_insts.append(st.ins)

        # join sentinel: all phase-B gathers depend on all phase-A stores
        sent_pool = ctx.enter_context(tc.tile_pool(name="sent", bufs=1))
        sent = sent_pool.tile([1, 1], f32, tag="sent")
        sj = nc.vector.memset(sent[:], 0.0)
        for st in store_insts:
            add_dep_helper(sj.ins, st, sync=True, reason="phaseA->B join")

        # ---------------- Phase B: gather / attention ----------------
        kvp = ctx.enter_context(tc.tile_pool(name="kvg", bufs=3))
        qp = ctx.enter_context(tc.tile_pool(name="qg", bufs=3))
        wp = ctx.enter_context(tc.tile_pool(name="work", bufs=2))
        fp = ctx.enter_context(tc.tile_pool(name="fin", bufs=2))
        psB = ctx.enter_context(tc.tile_pool(name="psB", bufs=2, space="PSUM"))
        psT = ctx.enter_context(tc.tile_pool(name="psT", bufs=2, space="PSUM"))

        # chunk lists per block-in-superblock
        blk_chunks = []
        for bb in range(sbb):
            fl0 = sbb * 2 * plo
            h0 = sbb * cpb_lo
            cl = list(range(bb * 2 * plo, (bb + 1) * 2 * plo)) + \
                 list(range(fl0 + bb * flo, fl0 + (bb + 1) * flo)) + \
                 list(range(h0 + bb * cpb_hi, h0 + (bb + 1) * cpb_hi))
            blk_chunks.append(cl)

        gq = [0]

        def nextq():
            q = gq[0] % NQ
            gq[0] += 1
            return q

        for s in range(nsb):
            kvg = kvp.tile([128, ch, 2 * D], f16, tag="kvg")
            half = (sbb * cpb_lo) // 2
            hw16 = half * BLK // 16
            for hh in range(2):   # kv-lo split into two gathers for overlap
                g1 = nc.gpsimd.dma_gather(
                    out_ap=kvg[:, hh * half:(hh + 1) * half, :],
                    in_ap=t_kv[0:lo_limit, :],
                    idxs_ap=c_kvlo[:, s * (wlo // 16) + hh * hw16:
                                   s * (wlo // 16) + (hh + 1) * hw16],
                    num_idxs=half * BLK, num_idxs_reg=half * BLK,
                    elem_size=2 * D, queue_num=nextq(), single_packet=SP)
                add_dep_helper(g1.ins, sj.ins, sync=True, reason="waitA")
            if cpb_hi:
                g2 = nc.gpsimd.dma_gather(
                    out_ap=kvg[:, sbb * cpb_lo:ch, :],
                    in_ap=t_kv[lo_limit:npad, :],
                    idxs_ap=c_kvhi[:, s * (whi // 16):(s + 1) * (whi // 16)],
                    num_idxs=whi, num_idxs_reg=whi, elem_size=2 * D,
                    queue_num=nextq(), single_packet=SP)
                add_dep_helper(g2.ins, sj.ins, sync=True, reason="waitA")
            qg = qp.tile([128, qcols, D], f16, tag="qg")
            g3 = nc.gpsimd.dma_gather(
                out_ap=qg[:, :, :],
                idxs_ap=c_qidx[:, s * (qcols * BLK // 16):
                               (s + 1) * (qcols * BLK // 16)],
                in_ap=t_q[:, :],
                num_idxs=qcols * BLK, num_idxs_reg=qcols * BLK, elem_size=D,
                queue_num=nextq(), single_packet=SP)
            add_dep_helper(g3.ins, sj.ins, sync=True, reason="waitA")

            # ---- fused per-superblock edge math ----
            P = wp.tile([128, ch, BLK], bf16, tag="P")
            nc.vector.tensor_tensor(
                P[:, :, :],
                c_iota[:].unsqueeze(1).to_broadcast([128, ch, BLK]),
                c_dstl[:, s * ch:(s + 1) * ch].unsqueeze(2)
                    .to_broadcast([128, ch, BLK]),
                Alu.is_equal)
            msge = wp.tile([128, ch, D + H], bf16, tag="msge")
            qk = msge[:, :, 0:D].bitcast(f16)   # reuse msge bytes for qk
            npair, nlp = sbb * 2 * plo, sbb * plo
            nc.vector.tensor_mul(
                qk[:, 0:npair, :].rearrange("p (u t) d -> p u t d", t=2),
                qg[:, 0:nlp, :].unsqueeze(2).to_broadcast([128, nlp, 2, D]),
                kvg[:, 0:npair, 0:D].rearrange("p (u t) d -> p u t d", t=2))
            nc.vector.tensor_mul(
                qk[:, npair:ch, :], qg[:, nlp:qcols, :],
                kvg[:, npair:ch, 0:D])
            s4 = wp.tile([128, ch, H], f32, tag="s4")
            nc.vector.tensor_reduce(
                s4[:], qk.rearrange("p c (h d) -> p c h d", h=H),
                axis=mybir.AxisListType.X, op=Alu.add)
            expb = wp.tile([128, ch, H], bf16, tag="expb")
            nc.scalar.activation(expb[:], s4[:], Act.Exp)
            nc.scalar.copy(msge[:, :, D:D + H], expb[:])
            nc.vector.tensor_mul(
                msge[:, :, 0:D].rearrange("p c (h d) -> p c h d", h=H),
                kvg[:, :, D:2 * D].bitcast(bf16)
                    .rearrange("p c (h d) -> p c h d", h=H),
                expb[:].unsqueeze(3).to_broadcast([128, ch, H, HD]))

            aggps = psB.tile([128, sbb, D + H], f32, tag="agg")
            for bb in range(sbb):
                cl = blk_chunks[bb]
                for ci, c in enumerate(cl):
                    nc.tensor.matmul(aggps[:, bb, :], P[:, c, :], msge[:, c, :],
                                     start=(ci == 0), stop=(ci == len(cl) - 1))

            # ---- epilogue (both blocks at once) ----
            row0 = s * sbb * BLK
            rd = fp.tile([128, sbb, H], f32, tag="rd")
            nc.vector.tensor_scalar(rd[:], aggps[:, :, D:D + H], 1e-30,
                                    None, Alu.add)
            nc.vector.reciprocal(rd[:], rd[:])
            aggn = fp.tile([128, sbb, D], f32, tag="aggn")
            nc.vector.tensor_tensor(
                aggn[:].rearrange("p b (h d) -> p b h d", h=H),
                aggps[:, :, 0:D].rearrange("p b (h d) -> p b h d", h=H),
                rd[:].unsqueeze(3).to_broadcast([128, sbb, H, HD]),
                Alu.mult)
            aT = fp.tile([128, sbb, D], f16, tag="aT")
            ops = psT.tile([128, sbb, D], f32, tag="ops")
            for b in range(sbb):
                aTp = psT.tile([128, D], f32, tag="aTp", name=f"aTp{s}_{b}")
                nc.tensor.transpose(aTp[:], aggn[:, b, :], c_ident[:])
                nc.scalar.copy(aT[:, b, :], aTp[:])
                nc.tensor.matmul(ops[:, b, :], aT[:, b, :], c_wout[:],
                                 start=True, stop=True)
            tmp = fp.tile([128, sbb, D], f32, tag="tmp")
            nc.vector.scalar_tensor_tensor(
                tmp[:], ops[:], 0.0,
                c_bias[:].unsqueeze(1).to_broadcast([128, sbb, D]),
                Alu.bypass, Alu.add)
            rl = fp.tile([128, sbb, D], f32, tag="rl")
            nc.scalar.activation(rl[:], tmp[:], Act.Relu)
            xb = fp.tile([128, sbb, D], f32, tag="xb")
            for b in range(sbb):
                nc.sync.dma_start(xb[:, b, :],
                                  t_xl[row0 + b * BLK:row0 + (b + 1) * BLK, :])
            fin = fp.tile([128, sbb, D], f32, tag="fin")
            nc.vector.tensor_add(fin[:], rl[:], xb[:])
            for b in range(sbb):
                nc.sync.dma_start(t_out[row0 + b * BLK:row0 + (b + 1) * BLK, :],
                                  fin[:, b, :])

    nc.compile()
    return nc


def _run_sim(nc, in_maps):
    from concourse.bass_interp import CoreSim
    outs = []
    for m in in_maps:
        sim = CoreSim(nc)
        for k, v in m.items():
            sim.tensor(k)[:] = v
        sim.simulate(check_with_hw=False)
        outs.append(np.array(sim.tensor("out")))
    return outs


def _run_hw(nc, in_maps, trace=False):
    from concourse import bass_utils
    res = bass_utils.run_bass_kernel_spmd(
        nc, in_maps, core_ids=list(range(len(in_maps))), trace=trace)
    outs = [r["out"] for r in res.results]
    return outs, res


def kernel_custom(inputs, ncores=8, nbc=50, sbb=2, lo_limit=LO_LIMIT,
                  mode="hw", trace=False):
    meta, in_maps = _prep(
        inputs["x"], inputs["edge_index"], inputs["Wt"], inputs["Ws"],
        inputs["Wc"], inputs["Wout"], inputs["bout"],
        ncores, nbc, sbb, lo_limit)
    nc = _build(meta)
    if mode == "sim":
        outs = _run_sim(nc, in_maps)
        res = None
    else:
        outs, res = _run_hw(nc, in_maps, trace=trace)
    rows = np.concatenate(outs, axis=0)
    full = np.empty_like(rows)
    full[meta["perm"]] = rows          # inverse block permutation
    full = full[:meta["n"]]
    return full.astype(np.float32), res


def kernel(**inputs):
    out, _ = kernel_custom(inputs, ncores=8, nbc=50, sbb=2, mode="hw")
    return out


# revision 36
# speedup vs baseline: 1.4143x; 1.4143x over previous
"""GAT message-passing kernel for 8 Trainium2 NeuronCores (Bass/Tile).

Strategy (edge-parallel with dst-routing, per the sharding hint):
  * Sort edges by destination node on the host; partition destination
    nodes into 8 contiguous, 128-aligned ranges (50 blocks of 128 nodes
    per core).  Each core owns the full segment-softmax + scatter-add
    for its dst range, so no cross-core collectives are needed.
  * Phase A (on device, replicated): project x -> [k||v] for all nodes
    into an HBM table (k stored fp16, v stored as bf16 bit-pattern in
    the same fp16-typed rows); project x -> q (fp16) for the core's
    local nodes.
  * Phase B (per 2-block superblock): dma_gather 512-byte k||v rows by
    edge src (lo/hi tables, int16 gather indices) and 256-byte q rows
    by edge dst.  All per-edge math is fused into a handful of wide
    per-superblock DVE/ACT instructions:
      P    = is_equal(iota_bcast, dstloc_bcast)          (one-hot, bf16)
      qk   = qg * kg                                     (fp16)
      s4   = reduce_d(qk)                                (f32 scores)
      e    = exp(s4)                                     (ACT, f32)
      msge = [v * e | e]                                 (bf16)
    then one PE matmul per 128-edge chunk accumulates agg/denom per
    block in PSUM.  Epilogue: agg/denom, transpose, @Wout + bias, relu,
    +x (f32), store.

The single Bass program is shared by all 8 cores (SPMD); everything
data-dependent (gather indices, dst-local ids) arrives as per-core
input tensors with uniform shapes.
"""

import math
import numpy as np
import ml_dtypes

# ----- problem constants (hardcoded per contest rules) -----
N = 50000
E = 800000
D = 128          # IN_DIM == OUT_DIM == HEADS*HEAD_DIM
H = 4
HD = 32
BLK = 128
LO_LIMIT = 32768  # int16 gather-index limit

BF16 = ml_dtypes.bfloat16
FP16 = np.float16


def _ceil_div(a, b):
    return (a + b - 1) // b


def _wrap16(stream_i16):
    """Pack a flat descriptor-index stream into the [128, n/16] SBUF layout
    dma_gather expects (idx i at [i%16, i//16], replicated to all 8
    16-partition groups)."""
    n = stream_i16.shape[0]
    assert n % 16 == 0
    a = stream_i16.reshape(n // 16, 16).T  # [16, n/16]
    return np.tile(a, (8, 1)).astype(np.int16)  # [128, n/16]


def _prep(x, edge_index, Wt, Ws, Wc, Wout, bout, ncores, nbc, sbb, lo_limit):
    """Host-side preprocessing: sort/route edges, build all per-core arrays.

    Returns (meta dict, list of per-core in_maps)."""
    npb = ncores * nbc              # total padded blocks
    npad = npb * BLK                # padded node count
    nnc = nbc * BLK                 # nodes per core
    nsb = nbc // sbb                # superblocks per core
    assert nbc % sbb == 0

    x = np.asarray(x, np.float32)
    n = x.shape[0]
    src = np.asarray(edge_index[0]).astype(np.int64)
    dst = np.asarray(edge_index[1]).astype(np.int64)

    # --- degree-balanced node->block assignment (cuts worst-case chunk
    # counts, i.e. gather-descriptor padding).  Output rows come back in
    # block order and are inverse-permuted on the host.  The lo stream is
    # q-PAIRED (2 edges of one dst share a q descriptor), so balance lo
    # CELLS = ceil(deg_lo/2); the hi stream stays per-edge. ---
    lo_cnt = np.bincount(dst[src < lo_limit], minlength=npad).astype(np.int64)
    hi_cnt = np.bincount(dst[src >= lo_limit], minlength=npad).astype(np.int64)
    pairs_n = lo_cnt // 2            # full q-shared pairs contributed
    left_n = lo_cnt % 2
    # hard caps: plo=ceil(avg-pairs), cpb_hi=ceil(avg-hi) worth of slots
    PCAP = _ceil_div(int(pairs_n.sum()), npb * BLK) * BLK
    HCAP = _ceil_div(int(hi_cnt.sum()), npb * BLK) * BLK
    node_order_desc = np.lexsort((-hi_cnt, -pairs_n))
    blk_pr = np.zeros(npb, np.int64)
    blk_hi = np.zeros(npb, np.int64)
    blk_lf = np.zeros(npb, np.int64)
    blk_n = np.zeros(npb, np.int64)
    blk_of = np.zeros(npad, np.int64)
    wp_ = 1.0 / max(pairs_n.sum() / npb, 1.0)
    wh = 1.0 / max(hi_cnt.sum() / npb, 1.0)
    for nd in node_order_desc:
        pnd, hnd, lnd = pairs_n[nd], hi_cnt[nd], left_n[nd]
        feas = ((blk_pr + pnd <= PCAP) & (blk_hi + hnd <= HCAP) &
                (blk_lf + lnd <= BLK) & (blk_n < BLK))
        sc = blk_pr * wp_ + blk_hi * wh
        if feas.any():
            sc = np.where(feas, sc, np.inf)
            b = int(np.argmin(sc))
        else:
            over = (np.maximum(blk_pr + pnd - PCAP, 0) +
                    np.maximum(blk_hi + hnd - HCAP, 0) +
                    np.where(blk_n < BLK, 0, 1 << 30))
            b = int(np.argmin(over * 1e6 + sc))
        blk_of[nd] = b
        blk_pr[b] += pnd
        blk_hi[b] += hnd
        blk_lf[b] += lnd
        blk_n[b] += 1
    # node list per block (block-major permutation)
    perm = np.argsort(blk_of * (npad + 1) + np.arange(npad), kind="stable")
    # local index of each node within its block
    loc_of = np.zeros(npad, np.int64)
    loc_of[perm] = np.arange(npad) % BLK

    eb = blk_of[dst]                 # block of each edge
    order = np.argsort(eb, kind="stable")
    src_s = src[order].astype(np.int32)
    dst_s = dst[order].astype(np.int32)
    eb_s = eb[order]
    bounds = np.searchsorted(eb_s, np.arange(npb + 1)).astype(np.int64)
    lo_list, hi_list = [], []   # per block: (src_idx, dst_local)
    for b in range(npb):
        s, e = bounds[b], bounds[b + 1]
        bs, bd = src_s[s:e], loc_of[dst_s[s:e]].astype(np.int32)
        m = bs < lo_limit
        lo_list.append((bs[m], bd[m]))
        hi_list.append((bs[~m] - lo_limit, bd[~m]))

    # lo stream layout: full pairs (2 edges of one dst share a q descriptor)
    # + a small flat section for odd leftovers; hi stream flat.  The q rows
    # are 512B [q | onehot(local id)] so the gather also delivers the
    # scatter one-hot column (no DVE P-build); pad slots point at a
    # dedicated zero-onehot row ZR.
    def _npairs(ld):
        if len(ld) == 0:
            return 0
        return int((np.bincount(ld) // 2).sum())

    def _nleft(ld):
        if len(ld) == 0:
            return 0
        return int((np.bincount(ld) % 2).sum())

    plo = _ceil_div(max(1, max(_npairs(a[1]) for a in lo_list)), BLK)
    flo = _ceil_div(max(_nleft(a[1]) for a in lo_list), BLK)
    cpb_lo = 2 * plo + flo          # lo chunks per block
    cpb_hi = max(_ceil_div(len(a[0]), BLK) for a in hi_list)
    ch = sbb * (cpb_lo + cpb_hi)    # chunks per superblock
    qcols = sbb * (plo + flo + cpb_hi)  # q-gather columns per superblock
    wlo = sbb * cpb_lo * BLK        # lo edges slots per superblock
    whi = sbb * cpb_hi * BLK
    fl0 = sbb * 2 * plo             # first flat-lo chunk
    h0 = sbb * cpb_lo               # first hi chunk

    # padded x / weights
    xpad = np.zeros((npad, D), np.float32)
    xpad[:n] = x
    xT16 = np.ascontiguousarray(xpad.T).astype(FP16)       # [D, npad] fp16
    Wskvc = np.ascontiguousarray(
        np.concatenate([np.asarray(Ws, np.float32), np.asarray(Wc, np.float32)],
                       axis=1)).astype(FP16)               # [D, 2D]
    iota = np.tile(np.arange(BLK, dtype=np.float32)[None, :], (BLK, 1)).astype(BF16)
    ident = np.eye(BLK, dtype=np.float32)
    bias_rep = np.tile(np.asarray(bout, np.float32)[None, :], (BLK, 1))

    in_maps = []
    for c in range(ncores):
        perm_core = perm[c * nnc:(c + 1) * nnc]
        kvlo = np.zeros((nsb, wlo), np.int16)
        kvhi = np.zeros((nsb, max(whi, 16)), np.int16)
        qidx = np.zeros((nsb, qcols * BLK), np.int16)
        dstl = np.full((nsb, ch * BLK), -1, np.int32)
        for s in range(nsb):
            for bb in range(sbb):
                gb = c * nbc + s * sbb + bb            # global block
                gloc = (gb - c * nbc) * BLK            # q-table row base
                (ls, ld), (hs, hd_) = lo_list[gb], hi_list[gb]
                # --- lo segment: full pairs + odd leftovers ---
                o = np.argsort(ld, kind="stable")
                ls_s, ld_s = ls[o], ld[o]
                if len(ld_s):
                    first = np.searchsorted(ld_s, ld_s)      # run starts
                    r = np.arange(len(ld_s)) - first         # pos in run
                    degs = np.bincount(ld_s)
                    pairs_pfx = np.concatenate([[0], np.cumsum(degs // 2)])
                    left_pfx = np.concatenate([[0], np.cumsum(degs % 2)])
                    is_pair = r < 2 * (degs[ld_s] // 2)
                    # paired edges
                    kp = pairs_pfx[ld_s[is_pair]] + r[is_pair] // 2
                    t = r[is_pair] % 2
                    u, p = kp // BLK, kp % BLK
                    slot = (bb * 2 * plo + 2 * u + t) * BLK + p
                    kvlo[s, slot] = ls_s[is_pair]
                    dstl[s, slot] = ld_s[is_pair]
                    qslot = (bb * plo + u) * BLK + p
                    qidx[s, qslot] = (gloc + ld_s[is_pair]).astype(np.int16)
                    # leftover edges (flat-lo section)
                    il = left_pfx[ld_s[~is_pair]]
                    fu, fp_ = il // BLK, il % BLK
                    slot = (fl0 + bb * flo + fu) * BLK + fp_
                    kvlo[s, slot] = ls_s[~is_pair]
                    dstl[s, slot] = ld_s[~is_pair]
                    qslot = (sbb * plo + bb * flo + fu) * BLK + fp_
                    qidx[s, qslot] = (gloc + ld_s[~is_pair]).astype(np.int16)
                # --- hi segment (flat, per-edge q) ---
                o = bb * cpb_hi * BLK
                kvhi[s, o:o + len(hs)] = hs
                do = (h0 + bb * cpb_hi) * BLK
                dstl[s, do:do + len(hd_)] = hd_
                qo = (sbb * (plo + flo) + bb * cpb_hi) * BLK
                qidx[s, qo:qo + len(hd_)] = (gloc + hd_).astype(np.int16)
        # wrap idx streams into dma_gather SBUF layout, concat over superblocks
        kvlo_w = np.concatenate([_wrap16(kvlo[s]) for s in range(nsb)], axis=1)
        kvhi_w = np.concatenate(
            [_wrap16(kvhi[s]) for s in range(nsb)], axis=1) if cpb_hi else \
            np.zeros((128, 16), np.int16)
        qidx_w = np.concatenate([_wrap16(qidx[s]) for s in range(nsb)], axis=1)
        dstl_w = np.ascontiguousarray(
            dstl.reshape(nsb * ch, BLK).T.astype(np.float32)).astype(BF16)

        in_maps.append({
            "dstloc": dstl_w,
            "iota": iota,
            "xT_full": xT16,
            "xT_local": np.ascontiguousarray(xT16[:, perm_core]),
            "x_local": np.ascontiguousarray(xpad[perm_core]),
            "kvlo_idx": np.ascontiguousarray(kvlo_w),
            "kvhi_idx": np.ascontiguousarray(kvhi_w),
            "q_idx": np.ascontiguousarray(qidx_w),
            "Wskvc": Wskvc,
            "Wt": np.ascontiguousarray(np.asarray(Wt, np.float32)).astype(FP16),
            "Wout": np.ascontiguousarray(np.asarray(Wout, np.float32)).astype(FP16),
            "identb": np.eye(BLK, dtype=np.float32).astype(BF16),
            "ident": ident,
            "bias_rep": bias_rep,
        })

    meta = dict(ncores=ncores, nbc=nbc, sbb=sbb, nsb=nsb, npb=npb, npad=npad,
                nnc=nnc, cpb_lo=cpb_lo, cpb_hi=cpb_hi, ch=ch, qcols=qcols,
                plo=plo, flo=flo, wlo=wlo, whi=whi, lo_limit=lo_limit, n=n,
                perm=perm)
    return meta, in_maps


def _build(meta):
    """Build the (single, SPMD-shared) Bass program."""
    from contextlib import ExitStack
    import concourse.bacc as bacc
    import concourse.mybir as mybir
    import concourse.tile as tile
    from concourse.tile import add_dep_helper

    f32 = mybir.dt.float32
    f16 = mybir.dt.float16
    bf16 = mybir.dt.bfloat16
    i16 = mybir.dt.int16
    Alu = mybir.AluOpType
    Act = mybir.ActivationFunctionType

    nbc, sbb, nsb = meta["nbc"], meta["sbb"], meta["nsb"]
    npb, npad, nnc = meta["npb"], meta["npad"], meta["nnc"]
    cpb_lo, cpb_hi, ch = meta["cpb_lo"], meta["cpb_hi"], meta["ch"]
    qcols, plo, flo = meta["qcols"], meta["plo"], meta["flo"]
    wlo, whi, lo_limit = meta["wlo"], meta["whi"], meta["lo_limit"]

    import os
    NQ = int(os.environ.get("K_NQUEUES", "1"))
    SP = bool(int(os.environ.get("K_SP", "0")))
    nc = bacc.Bacc("TRN2", target_bir_lowering=False, debug=False,
                   num_swdge_queues=NQ)

    t_xT = nc.dram_tensor("xT_full", [D, npad], f16, kind="ExternalInput")
    t_xTl = nc.dram_tensor("xT_local", [D, nnc], f16, kind="ExternalInput")
    t_xl = nc.dram_tensor("x_local", [nnc, D], f32, kind="ExternalInput")
    t_kvlo = nc.dram_tensor("kvlo_idx", [128, nsb * wlo // 16], i16,
                            kind="ExternalInput")
    t_kvhi = nc.dram_tensor("kvhi_idx", [128, max(nsb * whi // 16, 16)], i16,
                            kind="ExternalInput")
    t_qidx = nc.dram_tensor("q_idx", [128, nsb * qcols * BLK // 16], i16,
                            kind="ExternalInput")
    t_dstl = nc.dram_tensor("dstloc", [128, nsb * ch], bf16,
                            kind="ExternalInput")
    t_iota = nc.dram_tensor("iota", [BLK, BLK], bf16, kind="ExternalInput")
    t_wskvc = nc.dram_tensor("Wskvc", [D, 2 * D], f16, kind="ExternalInput")
    t_wt = nc.dram_tensor("Wt", [D, D], f16, kind="ExternalInput")
    t_wout = nc.dram_tensor("Wout", [D, D], f16, kind="ExternalInput")
    t_identb = nc.dram_tensor("identb", [BLK, BLK], bf16, kind="ExternalInput")
    t_ident = nc.dram_tensor("ident", [BLK, BLK], f32, kind="ExternalInput")
    t_bias = nc.dram_tensor("bias_rep", [BLK, BLK], f32, kind="ExternalInput")

    t_kv = nc.dram_tensor("kv_table", [npad, 2 * D], f16, kind="Internal")
    t_q = nc.dram_tensor("q_core", [nnc + 16, D], f16, kind="Internal")
    t_out = nc.dram_tensor("out", [nnc, D], f32, kind="ExternalOutput")

    store_insts = []

    with ExitStack() as ctx:
        tc = ctx.enter_context(tile.TileContext(nc))
        cpool = ctx.enter_context(tc.tile_pool(name="const", bufs=1))

        def load_const(t, shape, dtype):
            s = cpool.tile(shape, dtype, tag=t.name)
            nc.sync.dma_start(s[:], t[:])
            return s

        c_wskvc = load_const(t_wskvc, [D, 2 * D], f16)
        c_wt = load_const(t_wt, [D, D], f16)
        c_wout = load_const(t_wout, [D, D], f16)
        c_identb = load_const(t_identb, [BLK, BLK], bf16)
        c_ident = load_const(t_ident, [BLK, BLK], f32)
        c_bias = load_const(t_bias, [BLK, BLK], f32)
        c_kvlo = load_const(t_kvlo, list(t_kvlo.shape), i16)
        c_kvhi = load_const(t_kvhi, list(t_kvhi.shape), i16)
        c_qidx = load_const(t_qidx, list(t_qidx.shape), i16)
        c_dstl = load_const(t_dstl, list(t_dstl.shape), bf16)
        c_iota = load_const(t_iota, [BLK, BLK], bf16)

        # ---------------- Phase A: projections (fp16) ----------------
        # Big tiles / few DMAs: per iteration 2048 xT columns in one load,
        # 16 node-blocks of [k||v] out in ONE 1 MB store.
        with tc.tile_pool(name="pa", bufs=2) as pa, \
             tc.tile_pool(name="pa_ps", bufs=2, space="PSUM") as pa_ps:
            # q for local blocks first
            pos = 0
            sq = 0
            while pos < nnc:
                w = min(512, nnc - pos)
                nb = w // 128
                xq = pa.tile([128, 512], f16, tag="xq")
                nc.sync.dma_start(xq[:, 0:w], t_xTl[:, pos:pos + w])
                psq = pa_ps.tile([128, 512], f32, tag="qps")
                for b in range(nb):
                    nc.tensor.matmul(psq[:, b * 128:(b + 1) * 128],
                                     xq[:, b * 128:(b + 1) * 128],
                                     c_wt[:], start=True, stop=True)
                qsb = pa.tile([128, 4, D], f16, tag="qsb")
                if sq % 2 == 0:
                    nc.vector.tensor_copy(qsb[:, 0:nb, :], psq[:, 0:w])
                else:
                    nc.scalar.copy(qsb[:, 0:nb, :], psq[:, 0:w])
                st = nc.sync.dma_start(
                    t_q[pos:pos + w, :].rearrange("(b p) d -> p b d", b=nb),
                    qsb[:, 0:nb, :])
                store_insts.append(st.ins)
                pos += w
                sq += 1
            zt = pa.tile([16, D], f16, tag="zrow")
            nc.vector.memset(zt[:], 0.0)
            st = nc.sync.dma_start(t_q[nnc:nnc + 16, :], zt[:])
            store_insts.append(st.ins)

            # kv table for all nodes
            for g in range(npad // 2048):
                xa = pa.tile([128, 2048], f16, tag="xa")
                nc.sync.dma_start(xa[:], t_xT[:, g * 2048:(g + 1) * 2048])
                kvsb = pa.tile([128, 16, 256], f16, tag="kvsb")
                for k in range(4):   # quads of 128-node blocks
                    ps = pa_ps.tile([128, 4, 256], f32, tag="kvps")
                    for b in range(4):
                        nc.tensor.matmul(
                            ps[:, b, :],
                            xa[:, (4 * k + b) * BLK:(4 * k + b + 1) * BLK],
                            c_wskvc[:], start=True, stop=True)
                    kvv = kvsb[:, 4 * k:4 * k + 4, :].rearrange(
                        "p b (two d) -> p b two d", two=2)
                    psv = ps[:].rearrange("p b (two d) -> p b two d", two=2)
                    # k halves fp16 (DVE), v halves bf16 bit-pattern (ACT)
                    nc.vector.tensor_copy(kvv[:, :, 0, :], psv[:, :, 0, :])
                    nc.scalar.copy(kvv[:, :, 1, :].bitcast(bf16),
                                   psv[:, :, 1, :])
                st = nc.sync.dma_start(
                    t_kv[g * 2048:(g + 1) * 2048, :].rearrange(
                        "(b p) d -> p b d", b=16), kvsb[:])
                store_insts.append(st.ins)

        # join sentinel: all phase-B gathers depend on all phase-A stores
        sent_pool = ctx.enter_context(tc.tile_pool(name="sent", bufs=1))
        sent = sent_pool.tile([1, 1], f32, tag="sent")
        sj = nc.vector.memset(sent[:], 0.0)
        for st in store_insts:
            add_dep_helper(sj.ins, st, sync=True, reason="phaseA->B join")

        # ---------------- Phase B: gather / attention ----------------
        kvp = ctx.enter_context(tc.tile_pool(name="kvg", bufs=3))
        qp = ctx.enter_context(tc.tile_pool(name="qg", bufs=3))
        wp = ctx.enter_context(tc.tile_pool(name="work", bufs=2))
        fp = ctx.enter_context(tc.tile_pool(name="fin", bufs=2))
        psB = ctx.enter_context(tc.tile_pool(name="psB", bufs=2, space="PSUM"))
        psT = ctx.enter_context(tc.tile_pool(name="psT", bufs=2, space="PSUM"))

        # chunk lists per block-in-superblock
        blk_chunks = []
        for bb in range(sbb):
            fl0 = sbb * 2 * plo
            h0 = sbb * cpb_lo
            cl = list(range(bb * 2 * plo, (bb + 1) * 2 * plo)) + \
                 list(range(fl0 + bb * flo, fl0 + (bb + 1) * flo)) + \
                 list(range(h0 + bb * cpb_hi, h0 + (bb + 1) * cpb_hi))
            blk_chunks.append(cl)

        gq = [0]

        def nextq():
            q = gq[0] % NQ
            gq[0] += 1
            return q

        for s in range(nsb):
            kvg = kvp.tile([128, ch, 2 * D], f16, tag="kvg")
            half = (sbb * cpb_lo) // 2
            hw16 = half * BLK // 16
            for hh in range(2):   # kv-lo split into two gathers for overlap
                g1 = nc.gpsimd.dma_gather(
                    out_ap=kvg[:, hh * half:(hh + 1) * half, :],
                    in_ap=t_kv[0:lo_limit, :],
                    idxs_ap=c_kvlo[:, s * (wlo // 16) + hh * hw16:
                                   s * (wlo // 16) + (hh + 1) * hw16],
                    num_idxs=half * BLK, num_idxs_reg=half * BLK,
                    elem_size=2 * D, queue_num=nextq(), single_packet=SP)
                add_dep_helper(g1.ins, sj.ins, sync=True, reason="waitA")
            if cpb_hi:
                g2 = nc.gpsimd.dma_gather(
                    out_ap=kvg[:, sbb * cpb_lo:ch, :],
                    in_ap=t_kv[lo_limit:npad, :],
                    idxs_ap=c_kvhi[:, s * (whi // 16):(s + 1) * (whi // 16)],
                    num_idxs=whi, num_idxs_reg=whi, elem_size=2 * D,
                    queue_num=nextq(), single_packet=SP)
                add_dep_helper(g2.ins, sj.ins, sync=True, reason="waitA")
            qg = qp.tile([128, qcols, D], f16, tag="qg")
            g3 = nc.gpsimd.dma_gather(
                out_ap=qg[:, :, :],
                idxs_ap=c_qidx[:, s * (qcols * BLK // 16):
                               (s + 1) * (qcols * BLK // 16)],
                in_ap=t_q[:, :],
                num_idxs=qcols * BLK, num_idxs_reg=qcols * BLK, elem_size=D,
                queue_num=nextq(), single_packet=SP)
            add_dep_helper(g3.ins, sj.ins, sync=True, reason="waitA")

            # ---- fused per-superblock edge math ----
            P = wp.tile([128, ch, BLK], bf16, tag="P")
            nc.vector.tensor_tensor(
                P[:, :, :],
                c_iota[:].unsqueeze(1).to_broadcast([128, ch, BLK]),
                c_dstl[:, s * ch:(s + 1) * ch].unsqueeze(2)
                    .to_broadcast([128, ch, BLK]),
                Alu.is_equal)
            msge = wp.tile([128, ch, D + H], bf16, tag="msge")
            qk = msge[:, :, 0:D].bitcast(f16)   # reuse msge bytes for qk
            npair, nlp = sbb * 2 * plo, sbb * plo
            nc.vector.tensor_mul(
                qk[:, 0:npair, :].rearrange("p (u t) d -> p u t d", t=2),
                qg[:, 0:nlp, :].unsqueeze(2).to_broadcast([128, nlp, 2, D]),
                kvg[:, 0:npair, 0:D].rearrange("p (u t) d -> p u t d", t=2))
            nc.vector.tensor_mul(
                qk[:, npair:ch, :], qg[:, nlp:qcols, :],
                kvg[:, npair:ch, 0:D])
            s4 = wp.tile([128, ch, H], f32, tag="s4")
            nc.vector.tensor_reduce(
                s4[:], qk.rearrange("p c (h d) -> p c h d", h=H),
                axis=mybir.AxisListType.X, op=Alu.add)
            expb = wp.tile([128, ch, H], bf16, tag="expb")
            nc.scalar.activation(expb[:], s4[:], Act.Exp)
            nc.scalar.copy(msge[:, :, D:D + H], expb[:])
            nc.vector.tensor_mul(
                msge[:, :, 0:D].rearrange("p c (h d) -> p c h d", h=H),
                kvg[:, :, D:2 * D].bitcast(bf16)
                    .rearrange("p c (h d) -> p c h d", h=H),
                expb[:].unsqueeze(3).to_broadcast([128, ch, H, HD]))

            aggps = psB.tile([128, sbb, D + H], f32, tag="agg")
            for bb in range(sbb):
                cl = blk_chunks[bb]
                for ci, c in enumerate(cl):
                    nc.tensor.matmul(aggps[:, bb, :], P[:, c, :], msge[:, c, :],
                                     start=(ci == 0), stop=(ci == len(cl) - 1))

            # ---- epilogue (both blocks at once) ----
            row0 = s * sbb * BLK
            rd = fp.tile([128, sbb, H], f32, tag="rd")
            nc.vector.tensor_scalar(rd[:], aggps[:, :, D:D + H], 1e-30,
                                    None, Alu.add)
            nc.vector.reciprocal(rd[:], rd[:])
            aggn = fp.tile([128, sbb, D], f32, tag="aggn")
            nc.vector.tensor_tensor(
                aggn[:].rearrange("p b (h d) -> p b h d", h=H),
                aggps[:, :, 0:D].rearrange("p b (h d) -> p b h d", h=H),
                rd[:].unsqueeze(3).to_broadcast([128, sbb, H, HD]),
                Alu.mult)
            aT = fp.tile([128, sbb, D], f16, tag="aT")
            ops = psT.tile([128, sbb, D], f32, tag="ops")
            for b in range(sbb):
                aTp = psT.tile([128, D], f32, tag="aTp", name=f"aTp{s}_{b}")
                nc.tensor.transpose(aTp[:], aggn[:, b, :], c_ident[:])
                nc.scalar.copy(aT[:, b, :], aTp[:])
                nc.tensor.matmul(ops[:, b, :], aT[:, b, :], c_wout[:],
                                 start=True, stop=True)
            tmp = fp.tile([128, sbb, D], f32, tag="tmp")
            nc.vector.scalar_tensor_tensor(
                tmp[:], ops[:], 0.0,
                c_bias[:].unsqueeze(1).to_broadcast([128, sbb, D]),
                Alu.bypass, Alu.add)
            rl = fp.tile([128, sbb, D], f32, tag="rl")
            nc.scalar.activation(rl[:], tmp[:], Act.Relu)
            xb = fp.tile([128, sbb, D], f32, tag="xb")
            for b in range(sbb):
                nc.sync.dma_start(xb[:, b, :],
                                  t_xl[row0 + b * BLK:row0 + (b + 1) * BLK, :])
            fin = fp.tile([128, sbb, D], f32, tag="fin")
            nc.vector.tensor_add(fin[:], rl[:], xb[:])
            for b in range(sbb):
                nc.sync.dma_start(t_out[row0 + b * BLK:row0 + (b + 1) * BLK, :],
                                  fin[:, b, :])

    nc.compile()
    return nc


def _run_sim(nc, in_maps):
    from concourse.bass_interp import CoreSim
    outs = []
    for m in in_maps:
        sim = CoreSim(nc)
        for k, v in m.items():
            sim.tensor(k)[:] = v
        sim.simulate(check_with_hw=False)
        outs.append(np.array(sim.tensor("out")))
    return outs


def _run_hw(nc, in_maps, trace=False):
    from concourse import bass_utils
    res = bass_utils.run_bass_kernel_spmd(
        nc, in_maps, core_ids=list(range(len(in_maps))), trace=trace)
    outs = [r["out"] for r in res.results]
    return outs, res


def kernel_custom(inputs, ncores=8, nbc=50, sbb=2, lo_limit=LO_LIMIT,
                  mode="hw", trace=False):
    meta, in_maps = _prep(
        inputs["x"], inputs["edge_index"], inputs["Wt"], inputs["Ws"],
        inputs["Wc"], inputs["Wout"], inputs["bout"],
        ncores, nbc, sbb, lo_limit)
    nc = _build(meta)
    if mode == "sim":
        outs = _run_sim(nc, in_maps)
        res = None
    else:
        outs, res = _run_hw(nc, in_maps, trace=trace)
    rows = np.concatenate(outs, axis=0)
    full = np.empty_like(rows)
    full[meta["perm"]] = rows          # inverse block permutation
    full = full[:meta["n"]]
    return full.astype(np.float32), res


def kernel(**inputs):
    out, _ = kernel_custom(inputs, ncores=8, nbc=50, sbb=2, mode="hw")
    return out
